# revision 7
# baseline (speedup 1.0000x reference)
"""Multi-head attention (B=8, S=1024, E=1024, H=16) on 8 TRN2 NeuronCores.

Strategy: pure data-parallel over batch -- core c computes the full MHA for
batch element c (no collectives). Host pre-transposes activations/weights so
every matmul operand is already K-major, and casts matmul inputs to bf16.

Reference quirk replicated exactly: the causal mask zeroes (not -inf) the
scaled scores before softmax, so masked positions contribute exp(0)=1 to both
the softmax denominator and the attention-weighted sum of V. We compute the
strictly-lower-triangle part with matmuls and add the "masked mass"
analytically: a suffix-sum-of-V term (rank-1 per row-tile) plus the masked
count, both folded into the same PSUM accumulation via K=1 matmuls.

Final [S, B, E] result is the reference's raw reshape of [B, S, E] memory.
"""

import sys

sys.path.insert(0, "/opt/trn_rl_repo")

import numpy as np
import ml_dtypes

import concourse.bass as bass
import concourse.tile as tile
import concourse.mybir as mybir
from concourse import bacc
from concourse.bass_utils import run_bass_kernel_spmd

B, S, E, H = 8, 1024, 1024, 16
D = E // H  # 64
P = 128
NT = S // P  # 8 tiles of 128 along s
ET = E // P  # 8 tiles of 128 along e
HW = D + 1  # 65: per-head v columns + ones column (denominator)

F32 = mybir.dt.float32
BF16 = mybir.dt.bfloat16
BF = ml_dtypes.bfloat16

_cache = {}


def _chunks(c0, c1, step):
    out = []
    while c0 < c1:
        out.append((c0, min(c0 + step, c1)))
        c0 = min(c0 + step, c1)
    return out


def build_nc():
    nc = bacc.Bacc("TRN2", target_bir_lowering=False, debug=False, num_devices=B)

    xqT = nc.dram_tensor("xqT", [E, S], BF16, kind="ExternalInput").ap()
    xkT = nc.dram_tensor("xkT", [E, S], BF16, kind="ExternalInput").ap()
    xvT = nc.dram_tensor("xvT", [E, S], BF16, kind="ExternalInput").ap()
    wqT = nc.dram_tensor("wqT", [E, E], BF16, kind="ExternalInput").ap()
    wkT = nc.dram_tensor("wkT", [E, E], BF16, kind="ExternalInput").ap()
    wvT = nc.dram_tensor("wvT", [E, E], BF16, kind="ExternalInput").ap()
    bq_d = nc.dram_tensor("bq_t", [P, ET], F32, kind="ExternalInput").ap()
    bk_d = nc.dram_tensor("bk_t", [P, ET], F32, kind="ExternalInput").ap()
    bv_d = nc.dram_tensor("bv_r", [1, E], BF16, kind="ExternalInput").ap()
    esel_d = nc.dram_tensor("esel", [NT, NT * P], BF16, kind="ExternalInput").ap()
    out_d = nc.dram_tensor("out", [S, E], F32, kind="ExternalOutput").ap()

    with tile.TileContext(nc) as tc:
        from contextlib import ExitStack

        with ExitStack() as top:
            const = top.enter_context(tc.tile_pool(name="const", bufs=1))
            ones_t = const.tile([P, P], BF16)
            nc.vector.memset(ones_t[:], 1.0)
            zero_b = const.tile([P, 1], F32)
            nc.vector.memset(zero_b[:], 0.0)
            # step_t[:, jt, it] = 1 if it < jt  (suffix-sum weights per K-tile)
            step_t = const.tile([P, NT, NT], BF16)
            nc.vector.memset(step_t[:], 0.0)
            for jt in range(1, NT):
                nc.vector.memset(step_t[:, jt, 0:jt], 1.0)
            # esel[k, it, m] = 1 if k == it: K=8 matmul that broadcasts row it
            # of sufx_sb across all 128 output partitions
            esel = const.tile([NT, NT, P], BF16)
            nc.sync.dma_start(esel[:], esel_d.rearrange("k (i m) -> k i m", m=P))
            bq_sb = const.tile([P, ET], F32)
            nc.sync.dma_start(bq_sb[:], bq_d[:])
            bk_sb = const.tile([P, ET], F32)
            nc.sync.dma_start(bk_sb[:], bk_d[:])
            bv_sb = const.tile([1, E], BF16)
            nc.sync.dma_start(bv_sb[:], bv_d[:])

            qk_pool = top.enter_context(tc.tile_pool(name="qk", bufs=1))
            qT_sb = qk_pool.tile([P, ET, S], BF16)  # [o%128, o_tile, s]
            kT_sb = qk_pool.tile([P, ET, S], BF16)
            v_aug = qk_pool.tile([P, NT, H * HW], BF16)  # [s%128, s_tile, h*65+d]
            sufx_sb = qk_pool.tile([NT, H * HW], BF16)

            # ones columns of v_aug (softmax-denominator accumulators)
            nc.vector.memset(
                v_aug[:].rearrange("p s (h x) -> p s h x", x=HW)[:, :, :, D : D + 1],
                1.0,
            )

            # ---------------- projection phase ----------------
            with ExitStack() as proj:
                wx = proj.enter_context(tc.tile_pool(name="wx", bufs=1))

                def load_kmaj(name, dram):
                    t = wx.tile([P, ET, E], BF16, name=name)
                    nc.sync.dma_start(
                        t[:], dram.rearrange("(et p) o -> p et o", p=P)
                    )
                    return t

                wq_sb = load_kmaj("wq_sb", wqT)
                wk_sb = load_kmaj("wk_sb", wkT)
                wv_sb = load_kmaj("wv_sb", wvT)
                xq_sb = load_kmaj("xq_sb", xqT)
                xk_sb = load_kmaj("xk_sb", xkT)
                xv_sb = load_kmaj("xv_sb", xvT)

                pp = proj.enter_context(
                    tc.tile_pool(name="pp", bufs=4, space=bass.MemorySpace.PSUM)
                )

                # qT[o, s] and kT[o, s] = W @ x.T (+bias), scale 1/8 folded into q
                for w_sb, x_sb, dst, b_sb, scl in (
                    (wq_sb, xq_sb, qT_sb, bq_sb, 0.125),
                    (wk_sb, xk_sb, kT_sb, bk_sb, 1.0),
                ):
                    for ot in range(ET):
                        for sc in range(2):
                            ps = pp.tile([P, 512], F32, tag="pp")
                            for et in range(ET):
                                nc.tensor.matmul(
                                    ps[:],
                                    lhsT=w_sb[:, et, ot * P : (ot + 1) * P],
                                    rhs=x_sb[:, et, sc * 512 : (sc + 1) * 512],
                                    start=(et == 0),
                                    stop=(et == ET - 1),
                                )
                            nc.scalar.activation(
                                dst[:, ot, sc * 512 : (sc + 1) * 512],
                                ps[:],
                                mybir.ActivationFunctionType.Identity,
                                bias=b_sb[:, ot : ot + 1],
                                scale=scl,
                            )

                # v[s, o] = x @ Wv.T + bv  (bias via K=1 matmul of ones x bv)
                for st in range(NT):
                    for oc in range(2):
                        ps = pp.tile([P, 512], F32, tag="pp")
                        for et in range(ET):
                            nc.tensor.matmul(
                                ps[:],
                                lhsT=xv_sb[:, et, st * P : (st + 1) * P],
                                rhs=wv_sb[:, et, oc * 512 : (oc + 1) * 512],
                                start=(et == 0),
                                stop=False,
                            )
                        nc.tensor.matmul(
                            ps[:],
                            lhsT=ones_t[0:1, 0:P],
                            rhs=bv_sb[0:1, oc * 512 : (oc + 1) * 512],
                            start=False,
                            stop=True,
                        )
                        nc.scalar.activation(
                            v_aug[:, st, :].rearrange("p (h x) -> p h x", x=HW)[
                                :, oc * 8 : (oc + 1) * 8, 0:D
                            ],
                            ps[:].rearrange("p (h x) -> p h x", x=D),
                            mybir.ActivationFunctionType.Copy,
                        )

            # ---------------- suffix sums of v ----------------
            # sufx[it, c] = sum_{j >= 128*(it+1)} v_aug[j, c]
            with ExitStack() as sfs:
                sp = sfs.enter_context(
                    tc.tile_pool(name="sp", bufs=1, space=bass.MemorySpace.PSUM)
                )
                sps = sp.tile([NT, H * HW], F32)
                for c0, c1 in _chunks(0, H * HW, 512):
                    for jt in range(1, NT):
                        nc.tensor.matmul(
                            sps[:, c0:c1],
                            lhsT=step_t[:, jt, :],
                            rhs=v_aug[:, jt, c0:c1],
                            start=(jt == 1),
                            stop=(jt == NT - 1),
                        )
                for c0, c1 in _chunks(0, H * HW, 512):
                    nc.scalar.activation(
                        sufx_sb[:, c0:c1],
                        sps[:, c0:c1],
                        mybir.ActivationFunctionType.Copy,
                    )

            # ---------------- attention ----------------
            with ExitStack() as att:
                out_sb = att.enter_context(tc.tile_pool(name="outb", bufs=1)).tile(
                    [P, NT, E], F32
                )  # [i%128, i_tile, h*64+d]
                expp = att.enter_context(tc.tile_pool(name="expp", bufs=2))
                scp = att.enter_context(
                    tc.tile_pool(name="scp", bufs=4, space=bass.MemorySpace.PSUM)
                )
                outp = att.enter_context(
                    tc.tile_pool(name="outp", bufs=4, space=bass.MemorySpace.PSUM)
                )
                small = att.enter_context(tc.tile_pool(name="small", bufs=8))

                def do_scores(h):
                    """scoresT strips + exp (+1.0 fill over diag) -> expT (bf16)."""
                    po = (h % 2) * D
                    ot = h // 2
                    expT = expp.tile([P, NT, S], BF16, tag="expT", name=f"expT{h}")
                    for jt in range(NT):
                        i0 = P * jt
                        for c0, c1 in _chunks(i0, S, 512):
                            ps = scp.tile([P, 512], F32, tag="sc")
                            nc.tensor.matmul(
                                ps[:, 0 : c1 - c0],
                                lhsT=kT_sb[po : po + D, ot, jt * P : (jt + 1) * P],
                                rhs=qT_sb[po : po + D, ot, c0:c1],
                                start=True,
                                stop=True,
                            )
                            nc.scalar.activation(
                                expT[:, jt, c0:c1],
                                ps[:, 0 : c1 - c0],
                                mybir.ActivationFunctionType.Exp,
                                bias=zero_b[:],
                            )
                        # inside the diagonal block keep i>=j, else exp(0)=1
                        nc.gpsimd.affine_select(
                            out=expT[:, jt, i0 : i0 + P],
                            in_=expT[:, jt, i0 : i0 + P],
                            pattern=[[1, P]],
                            base=0,
                            channel_multiplier=-1,
                            compare_op=mybir.AluOpType.is_ge,
                            fill=1.0,
                        )
                    return expT

                def do_attnv(h, expT):
                    """out rows = (expT.T @ v_aug + suffix mass) / denominator."""
                    for it in range(NT):
                        ops = outp.tile([P, HW], F32, tag="o")
                        for jt in range(it + 1):
                            nc.tensor.matmul(
                                ops[:],
                                lhsT=expT[:, jt, it * P : (it + 1) * P],
                                rhs=v_aug[:, jt, h * HW : (h + 1) * HW],
                                start=(jt == 0),
                                stop=(jt == it and it == NT - 1),
                            )
                        if it < NT - 1:
                            nc.tensor.matmul(
                                ops[:],
                                lhsT=esel[:, it, :],
                                rhs=sufx_sb[:, h * HW : (h + 1) * HW],
                                start=False,
                                stop=True,
                            )
                        recip = small.tile([P, 1], F32, tag="r")
                        nc.vector.reciprocal(recip[:], ops[:, D : D + 1])
                        nc.vector.tensor_scalar_mul(
                            out_sb[:, it, h * D : (h + 1) * D],
                            ops[:, 0:D],
                            recip[:],
                        )

                prev = None
                for h in range(H):
                    expT = do_scores(h)
                    if prev is not None:
                        do_attnv(*prev)
                    prev = (h, expT)
                do_attnv(*prev)

                nc.sync.dma_start(
                    out_d.rearrange("(it p) e -> p it e", p=P), out_sb[:]
                )

    nc.compile()
    return nc


def kernel(queries, keys, values, Wq, bq, Wk, bk, Wv, bv):
    if "nc" not in _cache:
        _cache["nc"] = build_nc()
    nc = _cache["nc"]

    shared = {
        "wqT": np.ascontiguousarray(Wq.T).astype(BF),
        "wkT": np.ascontiguousarray(Wk.T).astype(BF),
        "wvT": np.ascontiguousarray(Wv.T).astype(BF),
        # q-side bias pre-scaled by 1/sqrt(D); folded into the q eviction
        "bq_t": np.ascontiguousarray((bq.astype(np.float32) / 8.0).reshape(ET, P).T),
        "bk_t": np.ascontiguousarray(bk.astype(np.float32).reshape(ET, P).T),
        "bv_r": bv.astype(BF).reshape(1, E),
        "esel": np.ascontiguousarray(
            np.repeat(np.eye(NT, dtype=BF), P, axis=1).reshape(NT, NT * P)
        ),
    }
    in_maps = []
    for b in range(B):
        m = dict(shared)
        m["xqT"] = np.ascontiguousarray(queries[b].T).astype(BF)
        m["xkT"] = np.ascontiguousarray(keys[b].T).astype(BF)
        m["xvT"] = np.ascontiguousarray(values[b].T).astype(BF)
        in_maps.append(m)

    res = run_bass_kernel_spmd(nc, in_maps, list(range(B)))
    out = np.stack([res.results[c]["out"] for c in range(B)])  # [B, S, E]
    return out.reshape(S, B, E).astype(np.float32)
